# revision 19
# baseline (speedup 1.0000x reference)
"""Trainium2 Bass kernel for EnhancedMambaMixer (B=2, L=1024, H=1024, D=2048, N=16, K=4, R=64).

Sharding: 8-way tensor-parallel over intermediate_size D (256 channels/core).
Each core computes its D-shard of in_proj/conv/scan and a partial out_proj;
per-batch in-kernel AllReduces (2 x 393KB) combine the x_proj partials.
Host sums the 16 out_proj partials.

v3: the two batches are pipelined as independent halves (the scan resets at
the batch boundary, so each half is self-contained).  Batch 1's front-end
(in_proj/conv/x_proj/AllReduce) executes under batch 0's scan phase, which
removes the serial AllReduce hole of v2.  Other changes vs v2:
  - n-outer scan loop: B/C broadcasts are shared by both d-tiles (half the
    broadcast DMA traffic), each broadcast split in 2 for queue parallelism
  - x*D folded into the PSUM y-accumulation as a diag(D) matmul
  - dBu multiplies alternate DVE/Pool; out_proj evictions alternate ACT/Pool
  - latency-critical DMAs (ar_in, dtlr, bc) chunked across queues
"""

import ml_dtypes
import numpy as np

# Problem constants (hardcoded; kernel.py must be self-contained).
B, L, H = 2, 1024, 1024
D = 2048
N = 16
K = 4
R = 64
NCORES = 8
DP = D // NCORES          # 256 channels per core
T = B * L                 # 2048 fused time steps
TB = L                    # 1024 per batch
NTB = TB // 512           # 512-col chunks per batch (2)
DT2 = DP // 128           # d-tiles per core (2)
KH = H // 128             # in_proj contraction tiles (8)

_CACHE = {}


def _build_module():
    import concourse.bacc as bacc
    import concourse.mybir as mybir
    import concourse.tile as tile

    f32 = mybir.dt.float32
    f32r = mybir.dt.float32r
    bf16 = mybir.dt.bfloat16
    fp16 = mybir.dt.float16
    Alu = mybir.AluOpType
    Act = mybir.ActivationFunctionType

    nc = bacc.Bacc(
        "TRN2",
        target_bir_lowering=False,
        debug=False,
        num_devices=NCORES,
    )

    # ---- I/O -------------------------------------------------------------
    hsT = nc.dram_tensor("hsT", [H, T], bf16, kind="ExternalInput").ap()
    winT = nc.dram_tensor("winT", [H, 2 * DP], bf16, kind="ExternalInput").ap()
    wxT = nc.dram_tensor("wxT", [DP, R + 2 * N], bf16, kind="ExternalInput").ap()
    wdtT = nc.dram_tensor("wdtT", [R, DP], bf16, kind="ExternalInput").ap()
    bdt = nc.dram_tensor("bdt", [DP, 1], f32, kind="ExternalInput").ap()
    negA = nc.dram_tensor("negA", [DP, N], f32, kind="ExternalInput").ap()
    convb = nc.dram_tensor("convb", [DP, 1], f32, kind="ExternalInput").ap()
    dpdiag = nc.dram_tensor("dpdiag", [DT2, 128, 128], bf16, kind="ExternalInput").ap()
    woutT = nc.dram_tensor("woutT", [DP, H], bf16, kind="ExternalInput").ap()
    eye_d = nc.dram_tensor("eye", [128, 128], bf16, kind="ExternalInput").ap()
    convdiag = nc.dram_tensor(
        "convdiag", [DT2 * K, 128, 128], bf16, kind="ExternalInput"
    ).ap()
    outT = [
        nc.dram_tensor(f"outT_part{i}", [H, T], bf16, kind="ExternalOutput").ap()
        for i in range(DT2)
    ]

    with tile.TileContext(nc) as tc:
        with (
            tc.tile_pool(name="persist", bufs=1) as pp,
            tc.tile_pool(name="dram", bufs=1, space="DRAM") as dp,
            tc.tile_pool(name="loop", bufs=2) as lp,
        ):
            # ---------------- persistent SBUF tiles ----------------------
            hsT_sb = [pp.tile([128, T], bf16, name=f"hsT{k}") for k in range(KH)]
            winT_sb = [
                pp.tile([128, 2 * DP], bf16, name=f"winT{k}") for k in range(KH)
            ]
            xpad = [
                [pp.tile([128, TB + K - 1], bf16, name=f"xpad{b}_{i}")
                 for i in range(DT2)]
                for b in range(B)
            ]
            x = [
                [pp.tile([128, TB], bf16, name=f"x{b}_{i}") for i in range(DT2)]
                for b in range(B)
            ]
            sgr = [
                [pp.tile([128, TB], bf16, name=f"sgr{b}_{i}") for i in range(DT2)]
                for b in range(B)
            ]
            sg = [
                [pp.tile([128, TB], bf16, name=f"sg{b}_{i}") for i in range(DT2)]
                for b in range(B)
            ]
            dt_t = [
                [pp.tile([128, TB], f32, name=f"dt{b}_{i}") for i in range(DT2)]
                for b in range(B)
            ]
            dtx = [
                [pp.tile([128, TB], bf16, name=f"dtx{b}_{i}") for i in range(DT2)]
                for b in range(B)
            ]
            yf = [
                [pp.tile([128, TB], bf16, name=f"yf{b}_{i}") for i in range(DT2)]
                for b in range(B)
            ]
            ssm_local = [
                pp.tile([R + 2 * N, TB], bf16, name=f"ssml{b}") for b in range(B)
            ]
            dtlr = [pp.tile([R, TB], bf16, name=f"dtlr{b}") for b in range(B)]
            eye_sb = pp.tile([128, 128], bf16, name="eye_sb")
            convdiag_sb = [
                [pp.tile([128, 128], bf16, name=f"cdiag{i}_{k}") for k in range(K)]
                for i in range(DT2)
            ]
            dpdiag_sb = [pp.tile([128, 128], bf16, name=f"dpd{i}") for i in range(DT2)]
            convb_sb = [pp.tile([128, 1], f32, name=f"convb_sb{i}") for i in range(DT2)]
            bdt_sb = [pp.tile([128, 1], f32, name=f"bdt_sb{i}") for i in range(DT2)]
            negA_sb = [pp.tile([128, N], f32, name=f"negA_sb{i}") for i in range(DT2)]
            wxT_sb = [
                pp.tile([128, R + 2 * N], bf16, name=f"wxT_sb{i}") for i in range(DT2)
            ]
            wdtT_sb = pp.tile([R, DP], bf16, name="wdtT_sb")
            woutT_sb = [pp.tile([128, H], bf16, name=f"woutT_sb{i}") for i in range(DT2)]

            # DRAM staging for the collectives
            wu_in = dp.tile([1, 64], bf16, name="wu_in")
            wu_out = dp.tile([1, 64], bf16, name="wu_out", addr_space="Shared")
            ar_in = [dp.tile([R + 2 * N, TB], bf16, name=f"ar_in{b}") for b in range(B)]
            ar_out = [
                dp.tile([R + 2 * N, TB], bf16, name=f"ar_out{b}", addr_space="Shared")
                for b in range(B)
            ]


            # warmup collective: absorbs the one-time CC handshake cost
            # before the real AllReduces need it
            wu_sb = pp.tile([1, 64], bf16, name="wu_sb")
            nc.gpsimd.memset(wu_sb[:], 0.0)
            nc.gpsimd.dma_start(wu_in[:], wu_sb[:])
            nc.gpsimd.collective_compute(
                "AllReduce",
                Alu.add,
                replica_groups=[list(range(NCORES))],
                ins=[wu_in[:]],
                outs=[wu_out[:]],
            )

            # ---------------- const + input loads -------------------------
            # critical-path loads (winT + batch-0 hsT) issue first on sync;
            # consts go through the gpsimd queue so issue time overlaps
            for k in range(KH):
                nc.sync.dma_start(winT_sb[k][:], winT[128 * k : 128 * (k + 1), :])
                nc.sync.dma_start(
                    hsT_sb[k][:, 0:512], hsT[128 * k : 128 * (k + 1), 0:512]
                )
            for k in range(KH):
                nc.sync.dma_start(
                    hsT_sb[k][:, 512:TB], hsT[128 * k : 128 * (k + 1), 512:TB]
                )
            for i in range(DT2):
                rs = slice(128 * i, 128 * (i + 1))
                for k in range(K):
                    nc.gpsimd.dma_start(convdiag_sb[i][k][:], convdiag[K * i + k, :, :])
                nc.gpsimd.dma_start(dpdiag_sb[i][:], dpdiag[i, :, :])
                nc.gpsimd.dma_start(convb_sb[i][:], convb[rs, :])
                nc.gpsimd.dma_start(bdt_sb[i][:], bdt[rs, :])
                nc.gpsimd.dma_start(negA_sb[i][:], negA[rs, :])
                nc.gpsimd.dma_start(wxT_sb[i][:], wxT[rs, :])
                nc.gpsimd.dma_start(woutT_sb[i][:], woutT[rs, :])
            nc.gpsimd.dma_start(wdtT_sb[:], wdtT)
            nc.gpsimd.dma_start(eye_sb[:], eye_d)
            for b in range(B):
                for i in range(DT2):
                    nc.gpsimd.memset(xpad[b][i][:, 0 : K - 1], 0.0)


            # ---------------- PSUM pools: front-end -----------------------
            fe_cm = tc.tile_pool(name="fe", bufs=2, space="PSUM")
            fe = fe_cm.__enter__()
            pj_cm = tc.tile_pool(name="pj", bufs=4, space="PSUM")
            pj = pj_cm.__enter__()

            def in_proj_group(b, half, m, evict):
                for t in range(NTB):
                    p = pj.tile([128, 512], f32, name="pj", tag="pj", bufs=4)
                    for k in range(KH):
                        nc.tensor.matmul(
                            p[:],
                            winT_sb[k][
                                :, 128 * (half * DT2 + m) : 128 * (half * DT2 + m + 1)
                            ],
                            hsT_sb[k][:, TB * b + 512 * t : TB * b + 512 * (t + 1)],
                            start=(k == 0),
                            stop=(k == KH - 1),
                        )
                    evict(t, p)

            def front_end(b):
                # in_proj x-half
                for m in range(DT2):
                    def ev(t, p, m=m):
                        nc.scalar.copy(
                            xpad[b][m][:, K - 1 + 512 * t : K - 1 + 512 * (t + 1)], p[:]
                        )
                    in_proj_group(b, 0, m, ev)
                # depthwise causal conv on PE + silu
                for i in range(DT2):
                    cps = fe.tile([128, TB], f32, name="cps", tag="fe", bufs=2)
                    for nt in range(NTB):
                        for k in range(K):
                            nc.tensor.matmul(
                                cps[:, 512 * nt : 512 * (nt + 1)],
                                convdiag_sb[i][k][:],
                                xpad[b][i][:, 512 * nt + k : 512 * nt + k + 512],
                                start=(k == 0),
                                stop=(k == K - 1),
                            )
                    nc.scalar.activation(
                        x[b][i][:], cps[:], Act.Silu, bias=convb_sb[i][:]
                    )
                # x_proj partial over this core's channels
                sp = fe.tile([R + 2 * N, TB], f32, name="sp", tag="fe", bufs=2)
                for t in range(NTB):
                    for kd in range(DT2):
                        nc.tensor.matmul(
                            sp[:, 512 * t : 512 * (t + 1)],
                            wxT_sb[kd][:],
                            x[b][kd][:, 512 * t : 512 * (t + 1)],
                            start=(kd == 0),
                            stop=(kd == DT2 - 1),
                        )
                nc.scalar.copy(ssm_local[b][:], sp[:])
                for j in range(4):
                    rs = slice(24 * j, 24 * (j + 1))
                    nc.sync.dma_start(ar_in[b][rs, :], ssm_local[b][rs, :])
                nc.gpsimd.collective_compute(
                    "AllReduce",
                    Alu.add,
                    replica_groups=[list(range(NCORES))],
                    ins=[ar_in[b][:]],
                    outs=[ar_out[b][:]],
                )

            def gates(b):
                for m in range(DT2):
                    def evg(t, p, b=b, m=m):
                        nc.scalar.copy(sgr[b][m][:, 512 * t : 512 * (t + 1)], p[:])
                    in_proj_group(b, 1, m, evg)

            front_end(0)
            # batch-1 hsT loads issue only now, so they don't queue ahead of
            # batch 0's AllReduce staging on the DMA queues
            for k in range(KH):
                nc.sync.dma_start(
                    hsT_sb[k][:, TB : 2 * TB], hsT[128 * k : 128 * (k + 1), TB : 2 * TB]
                )
            gates(0)           # PE filler while AllReduce 0 is in flight
            front_end(1)
            gates(1)
            # silu(gate) here: same ACT table as conv-silu, and sg is ready
            # long before the gating consumers
            for b in range(B):
                for i in range(DT2):
                    nc.scalar.activation(sg[b][i][:], sgr[b][i][:], Act.Silu)

            # ---------------- dt = softplus(W_dt @ dt_lr + b) -------------
            # dt pipeline for batch 0 only; batch 1's is deferred into
            # scan_phase(0) so its AllReduce-1 dependency cannot block the
            # sync/PE queues ahead of batch 0's scan work
            b = 0
            for j in range(2):
                rs = slice(32 * j, 32 * (j + 1))
                nc.sync.dma_start(dtlr[b][rs, :], ar_out[b][rs, :])
            for m in range(DT2):
                dt_ps = fe.tile([128, TB], f32, name="dt_ps", tag="fe", bufs=2)
                for t in range(NTB):
                    nc.tensor.matmul(
                        dt_ps[:, 512 * t : 512 * (t + 1)],
                        wdtT_sb[:, 128 * m : 128 * (m + 1)],
                        dtlr[b][:, 512 * t : 512 * (t + 1)],
                        start=True,
                        stop=True,
                    )
                # softplus(z) = ln(exp(z) + 1)
                nc.scalar.activation(
                    dt_t[b][m][:], dt_ps[:], Act.Exp, bias=bdt_sb[m][:]
                )
                nc.scalar.activation(dt_t[b][m][:], dt_t[b][m][:], Act.Ln, bias=1.0)
                nc.vector.tensor_mul(dtx[b][m][:], dt_t[b][m][:], x[b][m][:])

            pj_cm.__exit__(None, None, None)
            fe_cm.__exit__(None, None, None)

            # ---------------- per-batch scan + out_proj -------------------
            y_cm = tc.tile_pool(name="psY", bufs=2, space="PSUM")
            psy = y_cm.__enter__()
            po_cm = tc.tile_pool(name="psO", bufs=4, space="PSUM")
            pso = po_cm.__enter__()

            evict_flip = [0]

            def dt_pipeline_late(b):
                for j in range(2):
                    rs = slice(32 * j, 32 * (j + 1))
                    nc.sync.dma_start(dtlr[b][rs, :], ar_out[b][rs, :])
                for m in range(DT2):
                    for t in range(NTB):
                        sl = slice(512 * t, 512 * (t + 1))
                        dt_ps = pso.tile([128, 512], f32, name="dtl", tag="po", bufs=4)
                        nc.tensor.matmul(
                            dt_ps[:],
                            wdtT_sb[:, 128 * m : 128 * (m + 1)],
                            dtlr[b][:, sl],
                            start=True,
                            stop=True,
                        )
                        nc.scalar.activation(
                            dt_t[b][m][:, sl], dt_ps[:], Act.Exp, bias=bdt_sb[m][:]
                        )
                        nc.scalar.activation(
                            dt_t[b][m][:, sl], dt_t[b][m][:, sl], Act.Ln, bias=1.0
                        )
                    nc.vector.tensor_mul(dtx[b][m][:], dt_t[b][m][:], x[b][m][:])

            def out_proj_chunk(b, i, c, dve_evict=False):
                sl = slice(512 * c, 512 * (c + 1))
                for m in range(H // 128):
                    po = pso.tile([128, 512], f32, name="po", tag="po", bufs=4)
                    nc.tensor.matmul(
                        po[:],
                        woutT_sb[i][:, 128 * m : 128 * (m + 1)],
                        yf[b][i][:, sl],
                        start=True,
                        stop=True,
                    )
                    ot = lp.tile([128, 512], bf16, name="ot", tag="ot", bufs=6)
                    if dve_evict and m % 2 == 1:
                        nc.vector.tensor_copy(ot[:], po[:])
                    else:
                        nc.scalar.copy(ot[:], po[:])
                    dst = outT[i][
                        128 * m : 128 * (m + 1), TB * b + 512 * c : TB * b + 512 * (c + 1)
                    ]
                    if dve_evict:
                        nc.sync.dma_start(dst, ot[:])
                    elif evict_flip[0] % 2 == 0:
                        nc.scalar.dma_start(dst, ot[:])
                    else:
                        nc.gpsimd.dma_start(dst, ot[:])
                    evict_flip[0] += 1

            def scan_phase(b, deferred=()):
                deferred = list(deferred)
                y_ps = [
                    psy.tile([128, TB], f32, name=f"y{b}_{i}", tag="y", bufs=2)
                    for i in range(DT2)
                ]
                for n in range(N):
                    Bb = lp.tile([128, TB], bf16, name="Bb", tag="Bb", bufs=3)
                    Cb = lp.tile([128, TB], bf16, name="Cb", tag="Cb", bufs=3)
                    for hcol in range(2):
                        cs = slice(512 * hcol, 512 * (hcol + 1))
                        nc.sync.dma_start(
                            Bb[:, cs],
                            ar_out[b][R + n : R + n + 1, cs].to_broadcast([128, 512]),
                        )
                        nc.sync.dma_start(
                            Cb[:, cs],
                            ar_out[b][
                                R + N + n : R + N + n + 1, cs
                            ].to_broadcast([128, 512]),
                        )
                    for i in range(DT2):
                        # fp16 dA: 2-byte ACT fast path, ample mantissa for
                        # decay factors in (0, 1]
                        dA = lp.tile([128, TB], fp16, name="dA", tag="dA", bufs=3)
                        nc.scalar.activation(
                            dA[:], dt_t[b][i][:], Act.Exp,
                            scale=negA_sb[i][:, n : n + 1],
                        )
                        dBu = lp.tile([128, TB], bf16, name="dBu", tag="dBu", bufs=3)
                        nc.vector.tensor_tensor(
                            out=dBu[:], in0=dtx[b][i][:], in1=Bb[:], op=Alu.mult
                        )
                        h = lp.tile([128, TB], bf16, name="h", tag="h", bufs=2)
                        g = lp.tile([128, TB], bf16, name="g", tag="g", bufs=2)
                        if n < N - 1:
                            nc.vector.tensor_tensor_scan(
                                h[:], dA[:], dBu[:], 0.0, Alu.mult, Alu.add
                            )
                            nc.vector.tensor_mul(g[:], h[:], Cb[:])
                            for c in range(NTB):
                                sl = slice(512 * c, 512 * (c + 1))
                                nc.tensor.matmul(
                                    y_ps[i][:, sl], eye_sb[:], g[:, sl],
                                    start=(n == 0), stop=False,
                                )
                            # drip the previous batch's out_proj into this
                            # n-loop so its ACT evictions don't delay our dA
                            if deferred and n in (3, 6, 9, 12) and i == 1:
                                out_proj_chunk(*deferred.pop(0))
                            if b + 1 < B and n == 8 and i == 1:
                                dt_pipeline_late(b + 1)
                        else:
                            # final n: pipeline the tail per 512-col chunk
                            for c in range(NTB):
                                sl = slice(512 * c, 512 * (c + 1))
                                nc.vector.tensor_tensor_scan(
                                    h[:, sl], dA[:, sl], dBu[:, sl],
                                    0.0 if c == 0 else h[:, 512 * c - 1 : 512 * c],
                                    Alu.mult, Alu.add,
                                )
                                nc.vector.tensor_mul(g[:, sl], h[:, sl], Cb[:, sl])
                                nc.tensor.matmul(
                                    y_ps[i][:, sl], eye_sb[:], g[:, sl],
                                    start=False, stop=False,
                                )
                                # fold x*D into the accumulation and finalize
                                nc.tensor.matmul(
                                    y_ps[i][:, sl], dpdiag_sb[i][:], x[b][i][:, sl],
                                    start=False, stop=True,
                                )
                                nc.vector.tensor_mul(
                                    yf[b][i][:, sl], y_ps[i][:, sl], sg[b][i][:, sl]
                                )
                                if b == B - 1:
                                    # last batch: emit now, DVE helps evict
                                    out_proj_chunk(b, i, c, dve_evict=True)
                return [(b, i, c) for i in range(DT2) for c in range(NTB)]

            d0 = scan_phase(0)
            scan_phase(1, deferred=d0)

            po_cm.__exit__(None, None, None)
            y_cm.__exit__(None, None, None)

    nc.compile()
    return nc


def _get_module():
    if "nc" not in _CACHE:
        _CACHE["nc"] = _build_module()
    return _CACHE["nc"]


def _conv_diag(cw):
    out = np.zeros((DT2 * K, 128, 128), dtype=np.float32)
    for i in range(DT2):
        for k in range(K):
            np.fill_diagonal(out[K * i + k], cw[128 * i : 128 * (i + 1), k])
    return out.astype(ml_dtypes.bfloat16)


def _dp_diag(dparam):
    out = np.zeros((DT2, 128, 128), dtype=np.float32)
    for i in range(DT2):
        np.fill_diagonal(out[i], dparam[128 * i : 128 * (i + 1)])
    return out.astype(ml_dtypes.bfloat16)


def _shard_inputs(inputs):
    """Build the 8 per-core input maps (host-side transposes are free)."""
    hs = np.asarray(inputs["hidden_states"], dtype=np.float32)
    W_in = np.asarray(inputs["W_in"], dtype=np.float32)
    conv_w = np.asarray(inputs["conv_w"], dtype=np.float32)
    conv_b = np.asarray(inputs["conv_b"], dtype=np.float32)
    W_x = np.asarray(inputs["W_x"], dtype=np.float32)
    W_dt = np.asarray(inputs["W_dt"], dtype=np.float32)
    b_dt = np.asarray(inputs["b_dt"], dtype=np.float32)
    A_log = np.asarray(inputs["A_log"], dtype=np.float32)
    D_param = np.asarray(inputs["D_param"], dtype=np.float32)
    W_out = np.asarray(inputs["W_out"], dtype=np.float32)

    hsT = np.ascontiguousarray(hs.reshape(T, H).T)
    in_maps = []
    for c in range(NCORES):
        dc = slice(DP * c, DP * (c + 1))
        winT = np.ascontiguousarray(
            np.concatenate([W_in[dc], W_in[D + DP * c : D + DP * (c + 1)]], axis=0).T
        )
        in_maps.append(
            {
                "hsT": hsT.astype(ml_dtypes.bfloat16),
                "eye": np.eye(128, dtype=np.float32).astype(ml_dtypes.bfloat16),
                "winT": winT.astype(ml_dtypes.bfloat16),
                "wxT": np.ascontiguousarray(W_x[:, dc].T).astype(ml_dtypes.bfloat16),
                "wdtT": np.ascontiguousarray(W_dt[dc].T).astype(ml_dtypes.bfloat16),
                "bdt": np.ascontiguousarray(b_dt[dc][:, None]),
                "negA": np.ascontiguousarray(-np.exp(A_log[dc])),
                "convdiag": _conv_diag(conv_w[dc, 0, :]),
                "convb": np.ascontiguousarray(conv_b[dc][:, None]),
                "dpdiag": _dp_diag(D_param[dc]),
                "woutT": np.ascontiguousarray(W_out[:, dc].T).astype(
                    ml_dtypes.bfloat16
                ),
            }
        )
    return in_maps


def kernel(**inputs):
    from concourse import bass_utils

    nc = _get_module()
    in_maps = _shard_inputs(inputs)
    res = bass_utils.run_bass_kernel_spmd(
        nc, in_maps, core_ids=list(range(NCORES))
    )
    _CACHE["last_results"] = res
    acc = np.zeros((H, T), dtype=np.float32)
    for r in res.results:
        acc += r["outT_part0"].astype(np.float32)
        acc += r["outT_part1"].astype(np.float32)
    return np.ascontiguousarray(acc.T).reshape(B, L, H)


# revision 21
# speedup vs baseline: 1.0470x; 1.0470x over previous
"""Trainium2 Bass kernel for EnhancedMambaMixer (B=2, L=1024, H=1024, D=2048, N=16, K=4, R=64).

Sharding: 8-way tensor-parallel over intermediate_size D (256 channels/core).
Each core computes its D-shard of in_proj/conv/scan and a partial out_proj;
per-batch in-kernel AllReduces (2 x 393KB) combine the x_proj partials.
Host sums the 16 out_proj partials.

v3: the two batches are pipelined as independent halves (the scan resets at
the batch boundary, so each half is self-contained).  Batch 1's front-end
(in_proj/conv/x_proj/AllReduce) executes under batch 0's scan phase, which
removes the serial AllReduce hole of v2.  Other changes vs v2:
  - n-outer scan loop: B/C broadcasts are shared by both d-tiles (half the
    broadcast DMA traffic), each broadcast split in 2 for queue parallelism
  - x*D folded into the PSUM y-accumulation as a diag(D) matmul
  - dBu multiplies alternate DVE/Pool; out_proj evictions alternate ACT/Pool
  - latency-critical DMAs (ar_in, dtlr, bc) chunked across queues
"""

import ml_dtypes
import numpy as np

# Problem constants (hardcoded; kernel.py must be self-contained).
B, L, H = 2, 1024, 1024
D = 2048
N = 16
K = 4
R = 64
NCORES = 8
DP = D // NCORES          # 256 channels per core
T = B * L                 # 2048 fused time steps
TB = L                    # 1024 per batch
NTB = TB // 512           # 512-col chunks per batch (2)
DT2 = DP // 128           # d-tiles per core (2)
KH = H // 128             # in_proj contraction tiles (8)

_CACHE = {}


def _build_module():
    import concourse.bacc as bacc
    import concourse.mybir as mybir
    import concourse.tile as tile

    f32 = mybir.dt.float32
    f32r = mybir.dt.float32r
    bf16 = mybir.dt.bfloat16
    fp16 = mybir.dt.float16
    Alu = mybir.AluOpType
    Act = mybir.ActivationFunctionType

    nc = bacc.Bacc(
        "TRN2",
        target_bir_lowering=False,
        debug=False,
        num_devices=NCORES,
    )

    # ---- I/O -------------------------------------------------------------
    hsT = nc.dram_tensor("hsT", [H, T], bf16, kind="ExternalInput").ap()
    winT = nc.dram_tensor("winT", [H, 2 * DP], bf16, kind="ExternalInput").ap()
    wxT = nc.dram_tensor("wxT", [DP, R + 2 * N], bf16, kind="ExternalInput").ap()
    wdtT = nc.dram_tensor("wdtT", [R, DP], bf16, kind="ExternalInput").ap()
    bdt = nc.dram_tensor("bdt", [DP, 1], f32, kind="ExternalInput").ap()
    negA = nc.dram_tensor("negA", [DP, N], f32, kind="ExternalInput").ap()
    convb = nc.dram_tensor("convb", [DP, 1], f32, kind="ExternalInput").ap()
    dpdiag = nc.dram_tensor("dpdiag", [DT2, 128, 128], bf16, kind="ExternalInput").ap()
    woutT = nc.dram_tensor("woutT", [DP, H], bf16, kind="ExternalInput").ap()
    eye_d = nc.dram_tensor("eye", [128, 128], bf16, kind="ExternalInput").ap()
    convdiag = nc.dram_tensor(
        "convdiag", [DT2 * K, 128, 128], bf16, kind="ExternalInput"
    ).ap()
    outT = [
        nc.dram_tensor(f"outT_part{i}", [H, T], bf16, kind="ExternalOutput").ap()
        for i in range(DT2)
    ]

    with tile.TileContext(nc) as tc:
        with (
            tc.tile_pool(name="persist", bufs=1) as pp,
            tc.tile_pool(name="dram", bufs=1, space="DRAM") as dp,
            tc.tile_pool(name="loop", bufs=2) as lp,
        ):
            # ---------------- persistent SBUF tiles ----------------------
            hsT_sb = [pp.tile([128, T], bf16, name=f"hsT{k}") for k in range(KH)]
            winT_sb = [
                pp.tile([128, 2 * DP], bf16, name=f"winT{k}") for k in range(KH)
            ]
            xpad = [
                [pp.tile([128, TB + K - 1], bf16, name=f"xpad{b}_{i}")
                 for i in range(DT2)]
                for b in range(B)
            ]
            x = [
                [pp.tile([128, TB], bf16, name=f"x{b}_{i}") for i in range(DT2)]
                for b in range(B)
            ]
            sgr = [
                [pp.tile([128, TB], bf16, name=f"sgr{b}_{i}") for i in range(DT2)]
                for b in range(B)
            ]
            sg = [
                [pp.tile([128, TB], bf16, name=f"sg{b}_{i}") for i in range(DT2)]
                for b in range(B)
            ]
            dt_t = [
                [pp.tile([128, TB], f32, name=f"dt{b}_{i}") for i in range(DT2)]
                for b in range(B)
            ]
            dtx = [
                [pp.tile([128, TB], bf16, name=f"dtx{b}_{i}") for i in range(DT2)]
                for b in range(B)
            ]
            yf = [
                [pp.tile([128, TB], bf16, name=f"yf{b}_{i}") for i in range(DT2)]
                for b in range(B)
            ]
            ssm_local = [
                pp.tile([R + 2 * N, TB], bf16, name=f"ssml{b}") for b in range(B)
            ]
            dtlr = [pp.tile([R, TB], bf16, name=f"dtlr{b}") for b in range(B)]
            eye_sb = pp.tile([128, 128], bf16, name="eye_sb")
            convdiag_sb = [
                [pp.tile([128, 128], bf16, name=f"cdiag{i}_{k}") for k in range(K)]
                for i in range(DT2)
            ]
            dpdiag_sb = [pp.tile([128, 128], bf16, name=f"dpd{i}") for i in range(DT2)]
            convb_sb = [pp.tile([128, 1], f32, name=f"convb_sb{i}") for i in range(DT2)]
            bdt_sb = [pp.tile([128, 1], f32, name=f"bdt_sb{i}") for i in range(DT2)]
            negA_sb = [pp.tile([128, N], f32, name=f"negA_sb{i}") for i in range(DT2)]
            wxT_sb = [
                pp.tile([128, R + 2 * N], bf16, name=f"wxT_sb{i}") for i in range(DT2)
            ]
            wdtT_sb = pp.tile([R, DP], bf16, name="wdtT_sb")
            woutT_sb = [pp.tile([128, H], bf16, name=f"woutT_sb{i}") for i in range(DT2)]

            # DRAM staging for the collectives
            ar_in = [dp.tile([R + 2 * N, TB], bf16, name=f"ar_in{b}") for b in range(B)]
            ar_out = [
                dp.tile([R + 2 * N, TB], bf16, name=f"ar_out{b}", addr_space="Shared")
                for b in range(B)
            ]


            # ---------------- const + input loads -------------------------
            # critical-path loads (winT + batch-0 hsT) issue first on sync;
            # consts go through the gpsimd queue so issue time overlaps
            for k in range(KH):
                nc.sync.dma_start(winT_sb[k][:], winT[128 * k : 128 * (k + 1), :])
                nc.sync.dma_start(
                    hsT_sb[k][:, 0:512], hsT[128 * k : 128 * (k + 1), 0:512]
                )
            for k in range(KH):
                nc.sync.dma_start(
                    hsT_sb[k][:, 512:TB], hsT[128 * k : 128 * (k + 1), 512:TB]
                )
            for i in range(DT2):
                rs = slice(128 * i, 128 * (i + 1))
                for k in range(K):
                    nc.gpsimd.dma_start(convdiag_sb[i][k][:], convdiag[K * i + k, :, :])
                nc.gpsimd.dma_start(dpdiag_sb[i][:], dpdiag[i, :, :])
                nc.gpsimd.dma_start(convb_sb[i][:], convb[rs, :])
                nc.gpsimd.dma_start(bdt_sb[i][:], bdt[rs, :])
                nc.gpsimd.dma_start(negA_sb[i][:], negA[rs, :])
                nc.gpsimd.dma_start(wxT_sb[i][:], wxT[rs, :])
                nc.gpsimd.dma_start(woutT_sb[i][:], woutT[rs, :])
            nc.gpsimd.dma_start(wdtT_sb[:], wdtT)
            nc.gpsimd.dma_start(eye_sb[:], eye_d)
            for b in range(B):
                for i in range(DT2):
                    nc.gpsimd.memset(xpad[b][i][:, 0 : K - 1], 0.0)


            # ---------------- PSUM pools: front-end -----------------------
            fe_cm = tc.tile_pool(name="fe", bufs=2, space="PSUM")
            fe = fe_cm.__enter__()
            pj_cm = tc.tile_pool(name="pj", bufs=4, space="PSUM")
            pj = pj_cm.__enter__()

            def in_proj_group(b, half, m, evict):
                for t in range(NTB):
                    p = pj.tile([128, 512], f32, name="pj", tag="pj", bufs=4)
                    for k in range(KH):
                        nc.tensor.matmul(
                            p[:],
                            winT_sb[k][
                                :, 128 * (half * DT2 + m) : 128 * (half * DT2 + m + 1)
                            ],
                            hsT_sb[k][:, TB * b + 512 * t : TB * b + 512 * (t + 1)],
                            start=(k == 0),
                            stop=(k == KH - 1),
                        )
                    evict(t, p)

            def front_end(b):
                # in_proj x-half
                for m in range(DT2):
                    def ev(t, p, m=m):
                        nc.scalar.copy(
                            xpad[b][m][:, K - 1 + 512 * t : K - 1 + 512 * (t + 1)], p[:]
                        )
                    in_proj_group(b, 0, m, ev)
                # depthwise causal conv on PE + silu
                for i in range(DT2):
                    cps = fe.tile([128, TB], f32, name="cps", tag="fe", bufs=2)
                    for nt in range(NTB):
                        for k in range(K):
                            nc.tensor.matmul(
                                cps[:, 512 * nt : 512 * (nt + 1)],
                                convdiag_sb[i][k][:],
                                xpad[b][i][:, 512 * nt + k : 512 * nt + k + 512],
                                start=(k == 0),
                                stop=(k == K - 1),
                            )
                    nc.scalar.activation(
                        x[b][i][:], cps[:], Act.Silu, bias=convb_sb[i][:]
                    )
                # x_proj partial over this core's channels
                sp = fe.tile([R + 2 * N, TB], f32, name="sp", tag="fe", bufs=2)
                for t in range(NTB):
                    for kd in range(DT2):
                        nc.tensor.matmul(
                            sp[:, 512 * t : 512 * (t + 1)],
                            wxT_sb[kd][:],
                            x[b][kd][:, 512 * t : 512 * (t + 1)],
                            start=(kd == 0),
                            stop=(kd == DT2 - 1),
                        )
                nc.scalar.copy(ssm_local[b][:], sp[:])
                for j in range(4):
                    rs = slice(24 * j, 24 * (j + 1))
                    nc.sync.dma_start(ar_in[b][rs, :], ssm_local[b][rs, :])
                nc.gpsimd.collective_compute(
                    "AllReduce",
                    Alu.add,
                    replica_groups=[list(range(NCORES))],
                    ins=[ar_in[b][:]],
                    outs=[ar_out[b][:]],
                )

            def gates(b):
                for m in range(DT2):
                    def evg(t, p, b=b, m=m):
                        nc.scalar.copy(sgr[b][m][:, 512 * t : 512 * (t + 1)], p[:])
                    in_proj_group(b, 1, m, evg)

            front_end(0)
            # batch-1 hsT loads issue only now, so they don't queue ahead of
            # batch 0's AllReduce staging on the DMA queues
            for k in range(KH):
                nc.sync.dma_start(
                    hsT_sb[k][:, TB : 2 * TB], hsT[128 * k : 128 * (k + 1), TB : 2 * TB]
                )
            gates(0)           # PE filler while AllReduce 0 is in flight
            front_end(1)
            gates(1)
            # silu(gate) here: same ACT table as conv-silu, and sg is ready
            # long before the gating consumers
            for b in range(B):
                for i in range(DT2):
                    nc.scalar.activation(sg[b][i][:], sgr[b][i][:], Act.Silu)

            # ---------------- dt = softplus(W_dt @ dt_lr + b) -------------
            # dt pipeline for batch 0 only; batch 1's is deferred into
            # scan_phase(0) so its AllReduce-1 dependency cannot block the
            # sync/PE queues ahead of batch 0's scan work
            b = 0
            for j in range(2):
                rs = slice(32 * j, 32 * (j + 1))
                nc.sync.dma_start(dtlr[b][rs, :], ar_out[b][rs, :])
            # softplus(z) = ln(exp(z) + 1), two-pass so the exp/ln act-table
            # switch happens once, not per d-tile
            for m in range(DT2):
                dt_ps = fe.tile([128, TB], f32, name="dt_ps", tag="fe", bufs=2)
                for t in range(NTB):
                    nc.tensor.matmul(
                        dt_ps[:, 512 * t : 512 * (t + 1)],
                        wdtT_sb[:, 128 * m : 128 * (m + 1)],
                        dtlr[b][:, 512 * t : 512 * (t + 1)],
                        start=True,
                        stop=True,
                    )
                nc.scalar.activation(
                    dt_t[b][m][:], dt_ps[:], Act.Exp, bias=bdt_sb[m][:]
                )
            for m in range(DT2):
                nc.scalar.activation(dt_t[b][m][:], dt_t[b][m][:], Act.Ln, bias=1.0)
                nc.vector.tensor_mul(dtx[b][m][:], dt_t[b][m][:], x[b][m][:])

            pj_cm.__exit__(None, None, None)
            fe_cm.__exit__(None, None, None)

            # ---------------- per-batch scan + out_proj -------------------
            y_cm = tc.tile_pool(name="psY", bufs=2, space="PSUM")
            psy = y_cm.__enter__()
            po_cm = tc.tile_pool(name="psO", bufs=4, space="PSUM")
            pso = po_cm.__enter__()

            evict_flip = [0]

            def dt_pipeline_late(b):
                for j in range(2):
                    rs = slice(32 * j, 32 * (j + 1))
                    nc.sync.dma_start(dtlr[b][rs, :], ar_out[b][rs, :])
                for m in range(DT2):
                    for t in range(NTB):
                        sl = slice(512 * t, 512 * (t + 1))
                        dt_ps = pso.tile([128, 512], f32, name="dtl", tag="po", bufs=4)
                        nc.tensor.matmul(
                            dt_ps[:],
                            wdtT_sb[:, 128 * m : 128 * (m + 1)],
                            dtlr[b][:, sl],
                            start=True,
                            stop=True,
                        )
                        nc.scalar.activation(
                            dt_t[b][m][:, sl], dt_ps[:], Act.Exp, bias=bdt_sb[m][:]
                        )
                for m in range(DT2):
                    nc.scalar.activation(
                        dt_t[b][m][:], dt_t[b][m][:], Act.Ln, bias=1.0
                    )
                    nc.vector.tensor_mul(dtx[b][m][:], dt_t[b][m][:], x[b][m][:])

            def out_proj_chunk(b, i, c, dve_evict=False):
                sl = slice(512 * c, 512 * (c + 1))
                for m in range(H // 128):
                    po = pso.tile([128, 512], f32, name="po", tag="po", bufs=4)
                    nc.tensor.matmul(
                        po[:],
                        woutT_sb[i][:, 128 * m : 128 * (m + 1)],
                        yf[b][i][:, sl],
                        start=True,
                        stop=True,
                    )
                    ot = lp.tile([128, 512], bf16, name="ot", tag="ot", bufs=6)
                    if dve_evict and m % 2 == 1:
                        nc.vector.tensor_copy(ot[:], po[:])
                    else:
                        nc.scalar.copy(ot[:], po[:])
                    dst = outT[i][
                        128 * m : 128 * (m + 1), TB * b + 512 * c : TB * b + 512 * (c + 1)
                    ]
                    if dve_evict:
                        nc.sync.dma_start(dst, ot[:])
                    elif evict_flip[0] % 2 == 0:
                        nc.scalar.dma_start(dst, ot[:])
                    else:
                        nc.gpsimd.dma_start(dst, ot[:])
                    evict_flip[0] += 1

            def scan_phase(b, deferred=()):
                deferred = list(deferred)
                y_ps = [
                    psy.tile([128, TB], f32, name=f"y{b}_{i}", tag="y", bufs=2)
                    for i in range(DT2)
                ]
                for n in range(N):
                    Bb = lp.tile([128, TB], bf16, name="Bb", tag="Bb", bufs=3)
                    Cb = lp.tile([128, TB], bf16, name="Cb", tag="Cb", bufs=3)
                    for hcol in range(2):
                        cs = slice(512 * hcol, 512 * (hcol + 1))
                        nc.sync.dma_start(
                            Bb[:, cs],
                            ar_out[b][R + n : R + n + 1, cs].to_broadcast([128, 512]),
                        )
                        nc.sync.dma_start(
                            Cb[:, cs],
                            ar_out[b][
                                R + N + n : R + N + n + 1, cs
                            ].to_broadcast([128, 512]),
                        )
                    for i in range(DT2):
                        # fp16 dA: 2-byte ACT fast path, ample mantissa for
                        # decay factors in (0, 1]
                        dA = lp.tile([128, TB], fp16, name="dA", tag="dA", bufs=3)
                        nc.scalar.activation(
                            dA[:], dt_t[b][i][:], Act.Exp,
                            scale=negA_sb[i][:, n : n + 1],
                        )
                        dBu = lp.tile([128, TB], bf16, name="dBu", tag="dBu", bufs=3)
                        nc.vector.tensor_tensor(
                            out=dBu[:], in0=dtx[b][i][:], in1=Bb[:], op=Alu.mult
                        )
                        h = lp.tile([128, TB], bf16, name="h", tag="h", bufs=2)
                        g = lp.tile([128, TB], bf16, name="g", tag="g", bufs=2)
                        if n < N - 1:
                            nc.vector.tensor_tensor_scan(
                                h[:], dA[:], dBu[:], 0.0, Alu.mult, Alu.add
                            )
                            nc.vector.tensor_mul(g[:], h[:], Cb[:])
                            for c in range(NTB):
                                sl = slice(512 * c, 512 * (c + 1))
                                nc.tensor.matmul(
                                    y_ps[i][:, sl], eye_sb[:], g[:, sl],
                                    start=(n == 0), stop=False,
                                )
                            # drip the previous batch's out_proj into this
                            # n-loop so its ACT evictions don't delay our dA
                            if deferred and n in (3, 6, 9, 12) and i == 1:
                                out_proj_chunk(*deferred.pop(0))
                            if b + 1 < B and n == 8 and i == 1:
                                dt_pipeline_late(b + 1)
                        else:
                            # final n: pipeline the tail per 512-col chunk
                            for c in range(NTB):
                                sl = slice(512 * c, 512 * (c + 1))
                                nc.vector.tensor_tensor_scan(
                                    h[:, sl], dA[:, sl], dBu[:, sl],
                                    0.0 if c == 0 else h[:, 512 * c - 1 : 512 * c],
                                    Alu.mult, Alu.add,
                                )
                                nc.vector.tensor_mul(g[:, sl], h[:, sl], Cb[:, sl])
                                nc.tensor.matmul(
                                    y_ps[i][:, sl], eye_sb[:], g[:, sl],
                                    start=False, stop=False,
                                )
                                # fold x*D into the accumulation and finalize
                                nc.tensor.matmul(
                                    y_ps[i][:, sl], dpdiag_sb[i][:], x[b][i][:, sl],
                                    start=False, stop=True,
                                )
                                nc.vector.tensor_mul(
                                    yf[b][i][:, sl], y_ps[i][:, sl], sg[b][i][:, sl]
                                )
                                if b == B - 1:
                                    # last batch: emit now, DVE helps evict
                                    out_proj_chunk(b, i, c, dve_evict=True)
                return [(b, i, c) for i in range(DT2) for c in range(NTB)]

            d0 = scan_phase(0)
            scan_phase(1, deferred=d0)

            po_cm.__exit__(None, None, None)
            y_cm.__exit__(None, None, None)

    nc.compile()
    return nc


def _get_module():
    if "nc" not in _CACHE:
        _CACHE["nc"] = _build_module()
    return _CACHE["nc"]


def _conv_diag(cw):
    out = np.zeros((DT2 * K, 128, 128), dtype=np.float32)
    for i in range(DT2):
        for k in range(K):
            np.fill_diagonal(out[K * i + k], cw[128 * i : 128 * (i + 1), k])
    return out.astype(ml_dtypes.bfloat16)


def _dp_diag(dparam):
    out = np.zeros((DT2, 128, 128), dtype=np.float32)
    for i in range(DT2):
        np.fill_diagonal(out[i], dparam[128 * i : 128 * (i + 1)])
    return out.astype(ml_dtypes.bfloat16)


def _shard_inputs(inputs):
    """Build the 8 per-core input maps (host-side transposes are free)."""
    hs = np.asarray(inputs["hidden_states"], dtype=np.float32)
    W_in = np.asarray(inputs["W_in"], dtype=np.float32)
    conv_w = np.asarray(inputs["conv_w"], dtype=np.float32)
    conv_b = np.asarray(inputs["conv_b"], dtype=np.float32)
    W_x = np.asarray(inputs["W_x"], dtype=np.float32)
    W_dt = np.asarray(inputs["W_dt"], dtype=np.float32)
    b_dt = np.asarray(inputs["b_dt"], dtype=np.float32)
    A_log = np.asarray(inputs["A_log"], dtype=np.float32)
    D_param = np.asarray(inputs["D_param"], dtype=np.float32)
    W_out = np.asarray(inputs["W_out"], dtype=np.float32)

    hsT = np.ascontiguousarray(hs.reshape(T, H).T)
    in_maps = []
    for c in range(NCORES):
        dc = slice(DP * c, DP * (c + 1))
        winT = np.ascontiguousarray(
            np.concatenate([W_in[dc], W_in[D + DP * c : D + DP * (c + 1)]], axis=0).T
        )
        in_maps.append(
            {
                "hsT": hsT.astype(ml_dtypes.bfloat16),
                "eye": np.eye(128, dtype=np.float32).astype(ml_dtypes.bfloat16),
                "winT": winT.astype(ml_dtypes.bfloat16),
                "wxT": np.ascontiguousarray(W_x[:, dc].T).astype(ml_dtypes.bfloat16),
                "wdtT": np.ascontiguousarray(W_dt[dc].T).astype(ml_dtypes.bfloat16),
                "bdt": np.ascontiguousarray(b_dt[dc][:, None]),
                "negA": np.ascontiguousarray(-np.exp(A_log[dc])),
                "convdiag": _conv_diag(conv_w[dc, 0, :]),
                "convb": np.ascontiguousarray(conv_b[dc][:, None]),
                "dpdiag": _dp_diag(D_param[dc]),
                "woutT": np.ascontiguousarray(W_out[:, dc].T).astype(
                    ml_dtypes.bfloat16
                ),
            }
        )
    return in_maps


def kernel(**inputs):
    from concourse import bass_utils

    nc = _get_module()
    in_maps = _shard_inputs(inputs)
    res = bass_utils.run_bass_kernel_spmd(
        nc, in_maps, core_ids=list(range(NCORES))
    )
    _CACHE["last_results"] = res
    acc = np.zeros((H, T), dtype=np.float32)
    for r in res.results:
        acc += r["outT_part0"].astype(np.float32)
        acc += r["outT_part1"].astype(np.float32)
    return np.ascontiguousarray(acc.T).reshape(B, L, H)
